# revision 8
# baseline (speedup 1.0000x reference)
"""Trainium2 Bass kernel for CAMPseudoLabel.

Math (from the reference):
  cam    = relu(feature[:, 1:] / 96**3);  cam = cam * (cam > 0.2)
  pseudo = argmax(cam, axis=1) (first occurrence), int32
  upd    = masks + pseudo * (masks == 0)
  dom    = tiny 2-layer conv3d classifier + linear on dom_feat

Sharding: 8 cores = batch(2) x depth-quarters(4) of the 96^3 volume.
Per core: feature slice [13, 24, 96, 96] -> [13, 128, 1728] (channel 0 of the
original 14 is dropped by the reference before any use, so it is never sent).
The dom classifier is tiny and computed redundantly on every core; core 0's
copy is returned.
"""

import numpy as np

import concourse.bass as bass
import concourse.bacc as bacc
import concourse.tile as tile
from concourse import mybir
from concourse import masks as masks_lib
from concourse.bass_utils import run_bass_kernel_spmd

F32 = mybir.dt.float32
I32 = mybir.dt.int32
Alu = mybir.AluOpType
Act = mybir.ActivationFunctionType

P = 128                 # SBUF partitions
FT = 1728               # free size per core: 24*96*96 / 128
NCH = 13                # cam channels (original channels 1..13)
GAMMA = 0.2
SCALE = float(np.float32(1.0) / np.float32(96 ** 3))  # f32(1/VOL), as jnp computes it
N_CORES = 8


def build_program():
    nc = bacc.Bacc("TRN2", target_bir_lowering=False, debug=False)

    # --- DRAM I/O (per core) ---
    feat_d = nc.declare_dram_parameter("feat", [NCH, P, FT], F32, isOutput=False)
    masks_d = nc.declare_dram_parameter("masks", [P, FT], I32, isOutput=False)
    domf_d = nc.declare_dram_parameter("dom_feat", [512, 64], F32, isOutput=False)
    c1w_d = nc.declare_dram_parameter("conv1_w", [128, 6912], F32, isOutput=False)
    c1b_d = nc.declare_dram_parameter("conv1_b", [128, 1], F32, isOutput=False)
    c2w_d = nc.declare_dram_parameter("conv2_w", [64, 3456], F32, isOutput=False)
    c2b_d = nc.declare_dram_parameter("conv2_b", [64, 1], F32, isOutput=False)
    lw_d = nc.declare_dram_parameter("lin_w", [64, 1], F32, isOutput=False)
    lb_d = nc.declare_dram_parameter("lin_b", [1, 1], F32, isOutput=False)

    cam_d = nc.declare_dram_parameter("cam", [NCH, P, FT], F32, isOutput=True)
    pseudo_d = nc.declare_dram_parameter("pseudo", [P, FT], I32, isOutput=True)
    upd_d = nc.declare_dram_parameter("upd", [P, FT], I32, isOutput=True)
    dom_d = nc.declare_dram_parameter("dom", [2, 1], F32, isOutput=True)

    with tile.TileContext(nc) as tc:
        build_cam(nc, tc, feat_d, masks_d, cam_d, pseudo_d, upd_d)
        build_dom(nc, tc, domf_d, c1w_d, c1b_d, c2w_d, c2b_d, lw_d, lb_d, dom_d)
    nc.finalize()
    return nc


def build_cam(nc, tc, feat_d, masks_d, cam_d, pseudo_d, upd_d):
    with (
        tc.tile_pool(name="featp", bufs=3) as featp,
        tc.tile_pool(name="sp", bufs=3) as sp,
        tc.tile_pool(name="thp", bufs=3) as thp,
        tc.tile_pool(name="gtp", bufs=2) as gtp,
        tc.tile_pool(name="state", bufs=1) as state,
    ):
        # int32 per-partition constants 0..NCH-1 (column c == value c)
        consts = state.tile([P, NCH], I32)
        for c in range(NCH):
            nc.gpsimd.memset(consts[:, c : c + 1], c)

        best = state.tile([P, FT], F32)   # running max (doubles as th tile for c==0)
        idx = state.tile([P, FT], I32)    # running argmax
        nc.gpsimd.memset(idx[:], 0)

        for c in range(NCH):
            ft = featp.tile([P, FT], F32)
            # gpsimd SWDGE: slot-recycling loads need >1 sem wait, which the
            # HWDGE direct2d descriptor cannot encode
            nc.gpsimd.dma_start(out=ft[:], in_=feat_d[c])
            s = sp.tile([P, FT], F32)
            nc.scalar.activation(s[:], ft[:], Act.Copy, bias=0.0, scale=SCALE)
            th = best if c == 0 else thp.tile([P, FT], F32)
            # th = (s > gamma) * s   (thresholded cam channel)
            nc.vector.scalar_tensor_tensor(
                out=th[:], in0=s[:], scalar=GAMMA, in1=s[:],
                op0=Alu.is_gt, op1=Alu.mult,
            )
            nc.sync.dma_start(out=cam_d[c], in_=th[:])
            if c > 0:
                gt = gtp.tile([P, FT], I32)
                nc.vector.tensor_tensor(out=gt[:], in0=th[:], in1=best[:], op=Alu.is_gt)
                nc.vector.tensor_tensor(out=best[:], in0=best[:], in1=th[:], op=Alu.max)
                # idx = max(idx, gt * c): strictly-greater keeps first occurrence
                nc.vector.scalar_tensor_tensor(
                    out=idx[:], in0=gt[:], scalar=consts[:, c : c + 1], in1=idx[:],
                    op0=Alu.mult, op1=Alu.max,
                )

        nc.sync.dma_start(out=pseudo_d[:], in_=idx[:])

        mk = state.tile([P, FT], I32)
        nc.sync.dma_start(out=mk[:], in_=masks_d[:])
        contrib = state.tile([P, FT], I32)
        # contrib = (masks == 0) * idx
        nc.vector.scalar_tensor_tensor(
            out=contrib[:], in0=mk[:], scalar=consts[:, 0:1], in1=idx[:],
            op0=Alu.is_equal, op1=Alu.mult,
        )
        updt = state.tile([P, FT], I32)
        nc.vector.tensor_tensor(out=updt[:], in0=mk[:], in1=contrib[:], op=Alu.add)
        nc.sync.dma_start(out=upd_d[:], in_=updt[:])


def build_dom(nc, tc, domf_d, c1w_d, c1b_d, c2w_d, c2b_d, lw_d, lb_d, dom_d):
    """dom_classifier: conv3d(256->128,k3,p1) -> maxpool2 -> relu ->
    conv3d(128->64,k3,p1) -> maxpool2 -> relu -> flatten -> linear(64->1).
    Input spatial 4^3. Convs as 27 shifted matmuls over zero-padded volumes."""
    with (
        tc.tile_pool(name="dconst", bufs=1) as dconst,
        tc.tile_pool(name="dw", bufs=1) as dw,
        tc.tile_pool(name="dwt", bufs=4) as dwt,
        tc.tile_pool(name="dact", bufs=1) as dact,
        tc.tile_pool(name="dpsum", bufs=1, space="PSUM") as dpsum,
        tc.tile_pool(name="dpsum_tr", bufs=2, space="PSUM") as dpsum_tr,
    ):
        ident = dconst.tile([P, P], F32)
        masks_lib.make_identity(nc, ident[:])
        ones_t = dconst.tile([P, 1], F32)
        nc.gpsimd.memset(ones_t[:], 1.0)

        c1w_s = dw.tile([128, 6912], F32)
        nc.sync.dma_start(out=c1w_s[:], in_=c1w_d[:])
        c2w_s = dw.tile([64, 3456], F32)
        nc.sync.dma_start(out=c2w_s[:], in_=c2w_d[:])
        c1b_s = dw.tile([128, 1], F32)
        nc.sync.dma_start(out=c1b_s[:], in_=c1b_d[:])
        c2b_s = dw.tile([64, 1], F32)
        nc.sync.dma_start(out=c2b_s[:], in_=c2b_d[:])
        lw_s = dw.tile([64, 1], F32)
        nc.sync.dma_start(out=lw_s[:], in_=lw_d[:])

        # padded conv1 inputs: [128, 6^3] per (batch, ic-block)
        pad1 = {}
        for b in range(2):
            for icb in range(2):
                df = dact.tile([128, 64], F32, tag="df")
                nc.gpsimd.dma_start(
                    out=df[:], in_=domf_d[b * 256 + icb * 128 : b * 256 + icb * 128 + 128, :]
                )
                pt = dact.tile([128, 216], F32, tag=f"pad1_{b}_{icb}")
                nc.gpsimd.memset(pt[:], 0.0)
                pv = pt.rearrange("p (x y z) -> p x y z", x=6, y=6, z=6)
                nc.scalar.copy(
                    pv[:, 1:5, 1:5, 1:5],
                    df.rearrange("p (x y z) -> p x y z", x=4, y=4, z=4)[:],
                )
                pad1[(b, icb)] = pv

        c1w_v = c1w_s.rearrange("o (i k) -> o i k", k=27)
        psum_c1 = [
            dpsum.tile([128, 64], F32, tag=f"psc1_{b}", name=f"psc1_{b}") for b in range(2)
        ]
        n_acc = 0
        for icb in range(2):
            for k in range(27):
                pt = dpsum_tr.tile([128, 128], F32, tag="tr")
                nc.tensor.transpose(pt[:], c1w_v[:, icb * 128 : (icb + 1) * 128, k], ident[:])
                wt = dwt.tile([128, 128], F32, tag="w1t")
                nc.scalar.copy(wt[:], pt[:])
                dx, dy, dz = k // 9, (k // 3) % 3, k % 3
                for b in range(2):
                    nc.tensor.matmul(
                        psum_c1[b][:],
                        lhsT=wt[:],
                        rhs=pad1[(b, icb)][:, dx : dx + 4, dy : dy + 4, dz : dz + 4],
                        start=(n_acc == 0),
                        stop=(n_acc == 53),
                    )
                n_acc += 1

        # maxpool2 (on PSUM views) -> +bias -> relu -> padded conv2 input
        pad2 = {}
        for b in range(2):
            pool_v = psum_c1[b].rearrange(
                "p (x a y b2 z c) -> p x a y b2 z c", x=2, a=2, y=2, b2=2, z=2, c=2
            )
            mp = dact.tile([128, 8], F32, tag=f"mp1_{b}")
            first = True
            for da in range(2):
                for db in range(2):
                    for dc in range(2):
                        v = pool_v[:, :, da, :, db, :, dc]
                        if first:
                            nc.vector.tensor_copy(
                                mp.rearrange("p (x y z) -> p x y z", x=2, y=2, z=2)[:], v
                            )
                            first = False
                        else:
                            mv = mp.rearrange("p (x y z) -> p x y z", x=2, y=2, z=2)
                            nc.vector.tensor_tensor(out=mv[:], in0=mv[:], in1=v, op=Alu.max)
            relu1 = dact.tile([128, 8], F32, tag=f"relu1_{b}")
            nc.scalar.activation(relu1[:], mp[:], Act.Relu, bias=c1b_s[:, 0:1], scale=1.0)
            pt2 = dact.tile([128, 64], F32, tag=f"pad2_{b}")
            nc.gpsimd.memset(pt2[:], 0.0)
            pv2 = pt2.rearrange("p (x y z) -> p x y z", x=4, y=4, z=4)
            nc.scalar.copy(
                pv2[:, 1:3, 1:3, 1:3],
                relu1.rearrange("p (x y z) -> p x y z", x=2, y=2, z=2)[:],
            )
            pad2[b] = pv2

        c2w_v = c2w_s.rearrange("o (i k) -> o i k", k=27)
        psum_c2 = [
            dpsum.tile([64, 8], F32, tag=f"psc2_{b}", name=f"psc2_{b}") for b in range(2)
        ]
        n_acc = 0
        for k in range(27):
            pt = dpsum_tr.tile([128, 128], F32, tag="tr", name="tr2")[0:128, 0:64]
            nc.tensor.transpose(pt[:], c2w_v[:, :, k], ident[0:64, 0:64])
            wt = dwt.tile([128, 64], F32, tag="w2t")
            nc.scalar.copy(wt[:], pt[:])
            dx, dy, dz = k // 9, (k // 3) % 3, k % 3
            for b in range(2):
                nc.tensor.matmul(
                    psum_c2[b][:],
                    lhsT=wt[:],
                    rhs=pad2[b][:, dx : dx + 2, dy : dy + 2, dz : dz + 2],
                    start=(n_acc == 0),
                    stop=(n_acc == 26),
                )
            n_acc += 1

        # maxpool over all 8 voxels -> +bias -> relu -> x2[64, batch]
        xw = dact.tile([128, 2], F32, tag="xw")
        for b in range(2):
            mp2 = dact.tile([64, 1], F32, tag=f"mp2_{b}")
            nc.vector.tensor_reduce(
                out=mp2[:], in_=psum_c2[b][:], axis=mybir.AxisListType.X, op=Alu.max
            )
            x2 = dact.tile([64, 1], F32, tag=f"x2_{b}")
            nc.scalar.activation(x2[:], mp2[:], Act.Relu, bias=c2b_s[:, 0:1], scale=1.0)
            # xw[0:64, b] = x2 * lin_w
            nc.vector.tensor_tensor(
                out=xw[0:64, b : b + 1], in0=x2[:], in1=lw_s[:], op=Alu.mult
            )
        # bias row: xw[64, b] = lin_b
        nc.gpsimd.dma_start(out=xw[64:65, 0:1], in_=lb_d[:])
        nc.gpsimd.dma_start(out=xw[64:65, 1:2], in_=lb_d[:])

        psd = dpsum.tile([2, 1], F32, tag="psd")
        nc.tensor.matmul(psd[:], lhsT=xw[0:65, 0:2], rhs=ones_t[0:65, 0:1], start=True, stop=True)
        dom_s = dact.tile([2, 1], F32, tag="dom_s")
        nc.scalar.copy(dom_s[:], psd[:])
        nc.sync.dma_start(out=dom_d[:], in_=dom_s[:])


_NC_CACHE = None


def _get_nc():
    global _NC_CACHE
    if _NC_CACHE is None:
        _NC_CACHE = build_program()
    return _NC_CACHE


def make_in_maps(inputs):
    feature = np.ascontiguousarray(np.asarray(inputs["feature"], dtype=np.float32))
    masks = np.ascontiguousarray(np.asarray(inputs["masks"], dtype=np.int32))
    shared = {
        "dom_feat": np.asarray(inputs["dom_feat"], np.float32).reshape(512, 64),
        "conv1_w": np.asarray(inputs["conv1_w"], np.float32).reshape(128, 6912),
        "conv1_b": np.asarray(inputs["conv1_b"], np.float32).reshape(128, 1),
        "conv2_w": np.asarray(inputs["conv2_w"], np.float32).reshape(64, 3456),
        "conv2_b": np.asarray(inputs["conv2_b"], np.float32).reshape(64, 1),
        "lin_w": np.asarray(inputs["lin_w"], np.float32).reshape(64, 1),
        "lin_b": np.asarray(inputs["lin_b"], np.float32).reshape(1, 1),
    }
    in_maps = []
    for core in range(N_CORES):
        b, q = divmod(core, 4)
        dsl = slice(q * 24, (q + 1) * 24)
        in_maps.append(
            {
                "feat": np.ascontiguousarray(feature[b, 1:, dsl]).reshape(NCH, P, FT),
                "masks": np.ascontiguousarray(masks[b, 0, dsl]).reshape(P, FT),
                **shared,
            }
        )
    return in_maps


def assemble(results):
    cam = np.empty((2, NCH, 96, 96, 96), np.float32)
    pseudo = np.empty((2, 1, 96, 96, 96), np.int32)
    upd = np.empty((2, 1, 96, 96, 96), np.int32)
    for core in range(N_CORES):
        b, q = divmod(core, 4)
        dsl = slice(q * 24, (q + 1) * 24)
        r = results[core]
        cam[b, :, dsl] = np.asarray(r["cam"]).reshape(NCH, 24, 96, 96)
        pseudo[b, 0, dsl] = np.asarray(r["pseudo"]).reshape(24, 96, 96)
        upd[b, 0, dsl] = np.asarray(r["upd"]).reshape(24, 96, 96)
    dom = np.asarray(results[0]["dom"]).reshape(2, 1)
    return cam, pseudo, upd, dom


def _run(inputs, trace=False):
    nc = _get_nc()
    in_maps = make_in_maps(inputs)
    res = run_bass_kernel_spmd(nc, in_maps, list(range(N_CORES)), trace=trace)
    return assemble(res.results), res.exec_time_ns


def kernel(**inputs):
    out, _ = _run(inputs, trace=False)
    return out


# revision 14
# speedup vs baseline: 1.1949x; 1.1949x over previous
"""Trainium2 Bass kernel for CAMPseudoLabel.

Math (from the reference):
  cam    = relu(feature[:, 1:] / 96**3);  cam = cam * (cam > 0.2)
  pseudo = argmax(cam, axis=1) (first occurrence), int32
  upd    = masks + pseudo * (masks == 0)
  dom    = tiny 2-layer conv3d classifier + linear on dom_feat

Sharding: 8 cores = batch(2) x depth-quarters(4) of the 96^3 volume.
Per core: feature slice [13, 24, 96, 96] -> [13, 128, 1728] (channel 0 of the
original 14 is dropped by the reference before any use, so it is never sent).
The dom classifier is tiny; conv weights are pre-transposed on the host into
matmul (lhsT) layout and it is computed redundantly on every core, overlapped
with the CAM stream on the otherwise-idle tensor engine. Core 0's copy is
returned.
"""

import numpy as np

import concourse.bacc as bacc
import concourse.tile as tile
from concourse import mybir
from concourse.bass_utils import run_bass_kernel_spmd

F32 = mybir.dt.float32
I32 = mybir.dt.int32
Alu = mybir.AluOpType
Act = mybir.ActivationFunctionType

P = 128                 # SBUF partitions
FT = 1728               # free size per core: 24*96*96 / 128
NCH = 13                # cam channels (original channels 1..13)
GAMMA = 0.2
SCALE = float(np.float32(1.0) / np.float32(96 ** 3))  # f32(1/VOL), as jnp computes it
N_CORES = 8
# channels whose threshold op runs on gpsimd instead of DVE (load balance)
GP_TH_CHANNELS = frozenset()


def build_program():
    nc = bacc.Bacc("TRN2", target_bir_lowering=False, debug=False)

    # --- DRAM I/O (per core) ---
    feat_d = nc.declare_dram_parameter("feat", [NCH, P, FT], F32, isOutput=False)
    masks_d = nc.declare_dram_parameter("masks", [P, FT], I32, isOutput=False)
    domf_d = nc.declare_dram_parameter("dom_feat", [512, 64], F32, isOutput=False)
    # host-pretransposed conv weights, lhsT layout: [k, (icb,) ic, oc]
    c1wT_d = nc.declare_dram_parameter("conv1_wT", [27, 2, 128, 128], F32, isOutput=False)
    c1b_d = nc.declare_dram_parameter("conv1_b", [128, 1], F32, isOutput=False)
    c2wT_d = nc.declare_dram_parameter("conv2_wT", [27, 128, 64], F32, isOutput=False)
    c2b_d = nc.declare_dram_parameter("conv2_b", [64, 1], F32, isOutput=False)
    lw_d = nc.declare_dram_parameter("lin_w", [64, 1], F32, isOutput=False)
    lb_d = nc.declare_dram_parameter("lin_b", [1, 1], F32, isOutput=False)

    cam_d = nc.declare_dram_parameter("cam", [NCH, P, FT], F32, isOutput=True)
    pseudo_d = nc.declare_dram_parameter("pseudo", [P, FT], I32, isOutput=True)
    upd_d = nc.declare_dram_parameter("upd", [P, FT], I32, isOutput=True)
    dom_d = nc.declare_dram_parameter("dom", [2, 1], F32, isOutput=True)

    with tile.TileContext(nc) as tc:
        # dom first in program order: its PE/ACT work overlaps the CAM stream
        build_dom(nc, tc, domf_d, c1wT_d, c1b_d, c2wT_d, c2b_d, lw_d, lb_d, dom_d)
        build_cam(nc, tc, feat_d, masks_d, cam_d, pseudo_d, upd_d)
    nc.finalize()
    return nc


def build_cam(nc, tc, feat_d, masks_d, cam_d, pseudo_d, upd_d):
    BF16 = mybir.dt.bfloat16
    with (
        tc.tile_pool(name="featp", bufs=3) as featp,
        tc.tile_pool(name="sp", bufs=3) as sp,
        tc.tile_pool(name="thp", bufs=3) as thp,
        tc.tile_pool(name="kbp", bufs=3) as kbp,
        tc.tile_pool(name="gtp", bufs=2) as gtp,
        tc.tile_pool(name="state", bufs=1) as state,
    ):
        # bf16 per-partition constants 0..NCH-1 (column c == value c)
        consts = state.tile([P, NCH], BF16)
        for c in range(NCH):
            nc.gpsimd.memset(consts[:, c : c + 1], c)
        zero_i = state.tile([P, 1], I32)
        nc.gpsimd.memset(zero_i[:], 0)
        neg_gamma = state.tile([P, 1], F32)
        nc.gpsimd.memset(neg_gamma[:], -GAMMA)

        # The argmax runs on a bf16 key kb = bf16(relu(s - gamma)), which is an
        # order-isomorphic transform of the thresholded cam (exact ties
        # preserved; the downcast is monotone). bf16 doubles DVE throughput.
        idx = state.tile([P, FT], BF16)   # running argmax (values 0..12, exact)
        nc.gpsimd.memset(idx[:], 0)
        best = state.tile([P, FT], BF16)  # running key max

        for c in range(NCH):
            ft = featp.tile([P, FT], F32)
            nc.sync.dma_start(out=ft[:], in_=feat_d[c])
            s = sp.tile([P, FT], F32)
            nc.scalar.activation(s[:], ft[:], Act.Copy, bias=0.0, scale=SCALE)
            th = thp.tile([P, FT], F32)
            # th = (s > gamma) * s   (thresholded cam channel, exact f32)
            nc.vector.scalar_tensor_tensor(
                out=th[:], in0=s[:], scalar=GAMMA, in1=s[:],
                op0=Alu.is_gt, op1=Alu.mult,
            )
            nc.sync.dma_start(out=cam_d[c], in_=th[:])
            if c == 0:
                nc.scalar.activation(best[:], ft[:], Act.Relu, bias=neg_gamma[:, 0:1], scale=SCALE)
            else:
                kb = kbp.tile([P, FT], BF16)
                nc.scalar.activation(kb[:], ft[:], Act.Relu, bias=neg_gamma[:, 0:1], scale=SCALE)
                gt = gtp.tile([P, FT], BF16)
                nc.vector.tensor_tensor(out=gt[:], in0=kb[:], in1=best[:], op=Alu.is_gt)
                # idx = max(idx, gt * c): strictly-greater keeps first occurrence
                nc.vector.scalar_tensor_tensor(
                    out=idx[:], in0=gt[:], scalar=consts[:, c : c + 1], in1=idx[:],
                    op0=Alu.mult, op1=Alu.max,
                )
                if c < NCH - 1:
                    nc.vector.tensor_tensor(out=best[:], in0=best[:], in1=kb[:], op=Alu.max)

        pseudoi = state.tile([P, FT], I32)
        nc.scalar.copy(pseudoi[:], idx[:])
        nc.sync.dma_start(out=pseudo_d[:], in_=pseudoi[:])

        mk = state.tile([P, FT], I32)
        nc.sync.dma_start(out=mk[:], in_=masks_d[:])
        contrib = state.tile([P, FT], I32)
        # contrib = (masks == 0) * idx
        nc.vector.scalar_tensor_tensor(
            out=contrib[:], in0=mk[:], scalar=zero_i[:, 0:1], in1=pseudoi[:],
            op0=Alu.is_equal, op1=Alu.mult,
        )
        updt = state.tile([P, FT], I32)
        nc.vector.tensor_tensor(out=updt[:], in0=mk[:], in1=contrib[:], op=Alu.add)
        nc.sync.dma_start(out=upd_d[:], in_=updt[:])


def build_dom(nc, tc, domf_d, c1wT_d, c1b_d, c2wT_d, c2b_d, lw_d, lb_d, dom_d):
    """dom_classifier: conv3d(256->128,k3,p1) -> maxpool2 -> relu ->
    conv3d(128->64,k3,p1) -> maxpool2 -> relu -> flatten -> linear(64->1).
    Input spatial 4^3. Convs as 27 shifted matmuls over zero-padded volumes,
    both batches in one moving operand (N = 2*voxels)."""
    with (
        tc.tile_pool(name="dw", bufs=1) as dw,
        tc.tile_pool(name="dact", bufs=1) as dact,
        tc.tile_pool(name="ddf", bufs=4) as ddf,
        tc.tile_pool(name="dpsum", bufs=1, space="PSUM") as dpsum,
    ):
        ones_t = dw.tile([P, 1], F32)
        nc.gpsimd.memset(ones_t[:], 1.0)

        # weights: single DMA each, partition = input channel
        w1_s = dw.tile([128, 54 * 128], F32)
        nc.sync.dma_start(
            out=w1_s.rearrange("p (k i o) -> p k i o", k=27, i=2, o=128)[:],
            in_=c1wT_d.rearrange("k i p o -> p k i o"),
        )
        w2_s = dw.tile([128, 27 * 64], F32)
        nc.sync.dma_start(
            out=w2_s.rearrange("p (k o) -> p k o", k=27, o=64)[:],
            in_=c2wT_d.rearrange("k p o -> p k o"),
        )
        c1b_s = dw.tile([128, 1], F32)
        nc.sync.dma_start(out=c1b_s[:], in_=c1b_d[:])
        c2b_s = dw.tile([64, 1], F32)
        nc.sync.dma_start(out=c2b_s[:], in_=c2b_d[:])
        lw_s = dw.tile([64, 1], F32)
        nc.sync.dma_start(out=lw_s[:], in_=lw_d[:])

        # padded conv1 input: [128, 6^3] per (batch, ic-block)
        pad1 = {}
        for b in range(2):
            for icb in range(2):
                pt = dact.tile([128, 216], F32, tag=f"pad1_{b}_{icb}", name=f"pad1_{b}_{icb}")
                nc.gpsimd.memset(pt[:], 0.0)
                pv = pt.rearrange("p (x y z) -> p x y z", x=6, y=6, z=6)
                df = ddf.tile([128, 64], F32, tag="df", name=f"df_{b}_{icb}")
                nc.sync.dma_start(
                    out=df[:],
                    in_=domf_d[b * 256 + icb * 128 : b * 256 + icb * 128 + 128, :],
                )
                nc.scalar.copy(
                    pv[:, 1:5, 1:5, 1:5],
                    df.rearrange("p (x y z) -> p x y z", x=4, y=4, z=4)[:],
                )
                pad1[(b, icb)] = pv

        # conv1: accumulate 54 matmuls per batch into PSUM [oc=128, 4^3]
        # (rhs APs are limited to 3 free dims, so batches stay separate;
        # consecutive matmuls share the same lhsT to amortize weight loads)
        psum_c1 = [
            dpsum.tile([128, 64], F32, tag=f"psc1_{b}", name=f"psc1_{b}") for b in range(2)
        ]
        n_acc = 0
        for k in range(27):
            dx, dy, dz = k // 9, (k // 3) % 3, k % 3
            for icb in range(2):
                for b in range(2):
                    nc.tensor.matmul(
                        psum_c1[b][:],
                        lhsT=w1_s[:, (k * 2 + icb) * 128 : (k * 2 + icb + 1) * 128],
                        rhs=pad1[(b, icb)][:, dx : dx + 4, dy : dy + 4, dz : dz + 4],
                        start=(n_acc == 0),
                        stop=(n_acc == 53),
                    )
                n_acc += 1

        # maxpool2 (on PSUM views) -> +bias -> relu -> padded conv2 input
        pad2 = {}
        for b in range(2):
            pool_v = psum_c1[b].rearrange(
                "p (x a y b2 z c) -> p x a y b2 z c", x=2, a=2, y=2, b2=2, z=2, c=2
            )
            mp = dact.tile([128, 8], F32, tag=f"mp1_{b}", name=f"mp1_{b}")
            mv = mp.rearrange("p (x y z) -> p x y z", x=2, y=2, z=2)
            first = True
            for da in range(2):
                for db in range(2):
                    for dc in range(2):
                        v = pool_v[:, :, da, :, db, :, dc]
                        if first:
                            nc.vector.tensor_copy(mv[:], v)
                            first = False
                        else:
                            nc.vector.tensor_tensor(out=mv[:], in0=mv[:], in1=v, op=Alu.max)
            relu1 = dact.tile([128, 8], F32, tag=f"relu1_{b}", name=f"relu1_{b}")
            nc.scalar.activation(relu1[:], mp[:], Act.Relu, bias=c1b_s[:, 0:1], scale=1.0)
            pt2 = dact.tile([128, 64], F32, tag=f"pad2_{b}", name=f"pad2_{b}")
            nc.gpsimd.memset(pt2[:], 0.0)
            pv2 = pt2.rearrange("p (x y z) -> p x y z", x=4, y=4, z=4)
            nc.scalar.copy(
                pv2[:, 1:3, 1:3, 1:3],
                relu1.rearrange("p (x y z) -> p x y z", x=2, y=2, z=2)[:],
            )
            pad2[b] = pv2

        # conv2: accumulate 27 matmuls per batch into PSUM [oc=64, 2^3]
        psum_c2 = [
            dpsum.tile([64, 8], F32, tag=f"psc2_{b}", name=f"psc2_{b}") for b in range(2)
        ]
        n_acc = 0
        for k in range(27):
            dx, dy, dz = k // 9, (k // 3) % 3, k % 3
            for b in range(2):
                nc.tensor.matmul(
                    psum_c2[b][:],
                    lhsT=w2_s[:, k * 64 : (k + 1) * 64],
                    rhs=pad2[b][:, dx : dx + 2, dy : dy + 2, dz : dz + 2],
                    start=(n_acc == 0),
                    stop=(n_acc == 26),
                )
            n_acc += 1

        # maxpool over all 8 voxels -> +bias -> relu -> x2[64, batch]
        xw = dact.tile([128, 2], F32, tag="xw")
        for b in range(2):
            mp2 = dact.tile([64, 1], F32, tag=f"mp2_{b}", name=f"mp2_{b}")
            nc.vector.tensor_reduce(
                out=mp2[:], in_=psum_c2[b][:],
                axis=mybir.AxisListType.X, op=Alu.max,
            )
            x2 = dact.tile([64, 1], F32, tag=f"x2_{b}", name=f"x2_{b}")
            nc.scalar.activation(x2[:], mp2[:], Act.Relu, bias=c2b_s[:, 0:1], scale=1.0)
            # xw[0:64, b] = x2 * lin_w
            nc.vector.tensor_tensor(
                out=xw[0:64, b : b + 1], in0=x2[:], in1=lw_s[:], op=Alu.mult
            )
        # bias row: xw[64, b] = lin_b
        nc.gpsimd.dma_start(out=xw[64:65, 0:1], in_=lb_d[:])
        nc.gpsimd.dma_start(out=xw[64:65, 1:2], in_=lb_d[:])

        psd = dpsum.tile([2, 1], F32, tag="psd")
        nc.tensor.matmul(psd[:], lhsT=xw[0:65, 0:2], rhs=ones_t[0:65, 0:1], start=True, stop=True)
        dom_s = dact.tile([2, 1], F32, tag="dom_s")
        nc.scalar.copy(dom_s[:], psd[:])
        nc.sync.dma_start(out=dom_d[:], in_=dom_s[:])


_NC_CACHE = None


def _get_nc():
    global _NC_CACHE
    if _NC_CACHE is None:
        _NC_CACHE = build_program()
    return _NC_CACHE


def make_in_maps(inputs):
    feature = np.ascontiguousarray(np.asarray(inputs["feature"], dtype=np.float32))
    masks = np.ascontiguousarray(np.asarray(inputs["masks"], dtype=np.int32))
    c1w = np.asarray(inputs["conv1_w"], np.float32).reshape(128, 256, 27)
    c2w = np.asarray(inputs["conv2_w"], np.float32).reshape(64, 128, 27)
    shared = {
        "dom_feat": np.asarray(inputs["dom_feat"], np.float32).reshape(512, 64),
        # lhsT layout [k, icb, ic, oc] / [k, ic, oc]
        "conv1_wT": np.ascontiguousarray(c1w.transpose(2, 1, 0)).reshape(27, 2, 128, 128),
        "conv2_wT": np.ascontiguousarray(c2w.transpose(2, 1, 0)),
        "conv1_b": np.asarray(inputs["conv1_b"], np.float32).reshape(128, 1),
        "conv2_b": np.asarray(inputs["conv2_b"], np.float32).reshape(64, 1),
        "lin_w": np.asarray(inputs["lin_w"], np.float32).reshape(64, 1),
        "lin_b": np.asarray(inputs["lin_b"], np.float32).reshape(1, 1),
    }
    in_maps = []
    for core in range(N_CORES):
        b, q = divmod(core, 4)
        dsl = slice(q * 24, (q + 1) * 24)
        in_maps.append(
            {
                "feat": np.ascontiguousarray(feature[b, 1:, dsl]).reshape(NCH, P, FT),
                "masks": np.ascontiguousarray(masks[b, 0, dsl]).reshape(P, FT),
                **shared,
            }
        )
    return in_maps


def assemble(results):
    cam = np.empty((2, NCH, 96, 96, 96), np.float32)
    pseudo = np.empty((2, 1, 96, 96, 96), np.int32)
    upd = np.empty((2, 1, 96, 96, 96), np.int32)
    for core in range(N_CORES):
        b, q = divmod(core, 4)
        dsl = slice(q * 24, (q + 1) * 24)
        r = results[core]
        cam[b, :, dsl] = np.asarray(r["cam"]).reshape(NCH, 24, 96, 96)
        pseudo[b, 0, dsl] = np.asarray(r["pseudo"]).reshape(24, 96, 96)
        upd[b, 0, dsl] = np.asarray(r["upd"]).reshape(24, 96, 96)
    dom = np.asarray(results[0]["dom"]).reshape(2, 1)
    return cam, pseudo, upd, dom


def _run(inputs, trace=False):
    nc = _get_nc()
    in_maps = make_in_maps(inputs)
    res = run_bass_kernel_spmd(nc, in_maps, list(range(N_CORES)), trace=trace)
    return assemble(res.results), res.exec_time_ns


def kernel(**inputs):
    out, _ = _run(inputs, trace=False)
    return out


# revision 15
# speedup vs baseline: 1.7856x; 1.4943x over previous
"""Trainium2 Bass kernel for CAMPseudoLabel.

Math (from the reference):
  cam    = relu(feature[:, 1:] / 96**3);  cam = cam * (cam > 0.2)
  pseudo = argmax(cam, axis=1) (first occurrence), int32
  upd    = masks + pseudo * (masks == 0)
  dom    = tiny 2-layer conv3d classifier + linear on dom_feat

Sharding: 8 cores = batch(2) x depth-quarters(4) of the 96^3 volume.
Per core: feature slice [13, 24, 96, 96] -> [13, 128, 1728] (channel 0 of the
original 14 is dropped by the reference before any use, so it is never sent).
The dom classifier is tiny; conv weights are pre-transposed on the host into
matmul (lhsT) layout and it is computed redundantly on every core, overlapped
with the CAM stream on the otherwise-idle tensor engine. Core 0's copy is
returned.
"""

import numpy as np

import concourse.bacc as bacc
import concourse.tile as tile
from concourse import mybir
from concourse.bass_utils import run_bass_kernel_spmd

F32 = mybir.dt.float32
I32 = mybir.dt.int32
Alu = mybir.AluOpType
Act = mybir.ActivationFunctionType

P = 128                 # SBUF partitions
FT = 1728               # free size per core: 24*96*96 / 128
NCH = 13                # cam channels (original channels 1..13)
GAMMA = 0.2
SCALE = float(np.float32(1.0) / np.float32(96 ** 3))  # f32(1/VOL), as jnp computes it
N_CORES = 8
# channels whose threshold op runs on gpsimd instead of DVE (load balance)
GP_TH_CHANNELS = frozenset()


def build_program():
    nc = bacc.Bacc("TRN2", target_bir_lowering=False, debug=False)

    # --- DRAM I/O (per core) ---
    feat_d = nc.declare_dram_parameter("feat", [NCH, P, FT], F32, isOutput=False)
    masks_d = nc.declare_dram_parameter("masks", [P, FT], I32, isOutput=False)
    domf_d = nc.declare_dram_parameter("dom_feat", [512, 64], F32, isOutput=False)
    # host-pretransposed conv weights, lhsT layout: [k, (icb,) ic, oc]
    c1wT_d = nc.declare_dram_parameter("conv1_wT", [27, 2, 128, 128], F32, isOutput=False)
    c1b_d = nc.declare_dram_parameter("conv1_b", [128, 1], F32, isOutput=False)
    c2wT_d = nc.declare_dram_parameter("conv2_wT", [27, 128, 64], F32, isOutput=False)
    c2b_d = nc.declare_dram_parameter("conv2_b", [64, 1], F32, isOutput=False)
    lw_d = nc.declare_dram_parameter("lin_w", [64, 1], F32, isOutput=False)
    lb_d = nc.declare_dram_parameter("lin_b", [1, 1], F32, isOutput=False)

    cam_d = nc.declare_dram_parameter("cam", [NCH, P, FT], F32, isOutput=True)
    pseudo_d = nc.declare_dram_parameter("pseudo", [P, FT], I32, isOutput=True)
    upd_d = nc.declare_dram_parameter("upd", [P, FT], I32, isOutput=True)
    dom_d = nc.declare_dram_parameter("dom", [2, 1], F32, isOutput=True)

    from contextlib import ExitStack
    with tile.TileContext(nc) as tc, ExitStack() as ctx:
        # dom conv1 first in program order: PE work overlaps the CAM stream;
        # its DVE/ACT tail is emitted near the end of the CAM channel loop
        dom_st = build_dom_head(nc, tc, ctx, domf_d, c1wT_d, c1b_d, c2wT_d,
                                c2b_d, lw_d, lb_d)
        build_cam(nc, tc, feat_d, masks_d, cam_d, pseudo_d, upd_d,
                  dom_tail=lambda: build_dom_tail(nc, tc, dom_st, dom_d))
    nc.finalize()
    return nc


def build_cam(nc, tc, feat_d, masks_d, cam_d, pseudo_d, upd_d, dom_tail=None):
    BF16 = mybir.dt.bfloat16
    with (
        tc.tile_pool(name="featp", bufs=3) as featp,
        tc.tile_pool(name="sp", bufs=3) as sp,
        tc.tile_pool(name="thp", bufs=3) as thp,
        tc.tile_pool(name="kbp", bufs=3) as kbp,
        tc.tile_pool(name="gtp", bufs=2) as gtp,
        tc.tile_pool(name="state", bufs=1) as state,
    ):
        # bf16 per-partition constants 0..NCH-1 (column c == value c)
        consts = state.tile([P, NCH], BF16)
        for c in range(NCH):
            nc.gpsimd.memset(consts[:, c : c + 1], c)
        zero_i = state.tile([P, 1], I32)
        nc.gpsimd.memset(zero_i[:], 0)
        neg_gamma = state.tile([P, 1], F32)
        nc.gpsimd.memset(neg_gamma[:], -GAMMA)

        # The argmax runs on a bf16 key kb = bf16(relu(s - gamma)), which is an
        # order-isomorphic transform of the thresholded cam (exact ties
        # preserved; the downcast is monotone). bf16 doubles DVE throughput.
        idx = state.tile([P, FT], BF16)   # running argmax (values 0..12, exact)
        nc.gpsimd.memset(idx[:], 0)
        best = state.tile([P, FT], BF16)  # running key max

        for c in range(NCH):
            ft = featp.tile([P, FT], F32)
            nc.sync.dma_start(out=ft[:], in_=feat_d[c])
            s = sp.tile([P, FT], F32)
            nc.scalar.activation(s[:], ft[:], Act.Copy, bias=0.0, scale=SCALE)
            th = thp.tile([P, FT], F32)
            # th = (s > gamma) * s   (thresholded cam channel, exact f32)
            nc.vector.scalar_tensor_tensor(
                out=th[:], in0=s[:], scalar=GAMMA, in1=s[:],
                op0=Alu.is_gt, op1=Alu.mult,
            )
            nc.sync.dma_start(out=cam_d[c], in_=th[:])
            if c == 0:
                nc.scalar.activation(best[:], ft[:], Act.Relu, bias=neg_gamma[:, 0:1], scale=SCALE)
            else:
                kb = kbp.tile([P, FT], BF16)
                nc.scalar.activation(kb[:], ft[:], Act.Relu, bias=neg_gamma[:, 0:1], scale=SCALE)
                gt = gtp.tile([P, FT], BF16)
                nc.vector.tensor_tensor(out=gt[:], in0=kb[:], in1=best[:], op=Alu.is_gt)
                # idx = max(idx, gt * c): strictly-greater keeps first occurrence
                nc.vector.scalar_tensor_tensor(
                    out=idx[:], in0=gt[:], scalar=consts[:, c : c + 1], in1=idx[:],
                    op0=Alu.mult, op1=Alu.max,
                )
                if c < NCH - 1:
                    nc.vector.tensor_tensor(out=best[:], in0=best[:], in1=kb[:], op=Alu.max)
            if c == NCH - 2 and dom_tail is not None:
                dom_tail()

        pseudoi = state.tile([P, FT], I32)
        nc.scalar.copy(pseudoi[:], idx[:])
        nc.sync.dma_start(out=pseudo_d[:], in_=pseudoi[:])

        mk = state.tile([P, FT], I32)
        nc.sync.dma_start(out=mk[:], in_=masks_d[:])
        contrib = state.tile([P, FT], I32)
        # contrib = (masks == 0) * idx
        nc.vector.scalar_tensor_tensor(
            out=contrib[:], in0=mk[:], scalar=zero_i[:, 0:1], in1=pseudoi[:],
            op0=Alu.is_equal, op1=Alu.mult,
        )
        updt = state.tile([P, FT], I32)
        nc.vector.tensor_tensor(out=updt[:], in0=mk[:], in1=contrib[:], op=Alu.add)
        nc.sync.dma_start(out=upd_d[:], in_=updt[:])


def build_dom_head(nc, tc, ctx, domf_d, c1wT_d, c1b_d, c2wT_d, c2b_d, lw_d, lb_d):
    """Loads + padded inputs + conv1 matmuls (PE only; no DVE/ACT ops that
    could block the CAM stream). Batch goes innermost in the padded volumes so
    (z, b) merge into one AP dim and both batches share each matmul."""
    st = {}
    dw = ctx.enter_context(tc.tile_pool(name="dw", bufs=1))
    dact = ctx.enter_context(tc.tile_pool(name="dact", bufs=1))
    ddf = ctx.enter_context(tc.tile_pool(name="ddf", bufs=4))
    dpsum = ctx.enter_context(tc.tile_pool(name="dpsum", bufs=1, space="PSUM"))
    st["dact"], st["dpsum"] = dact, dpsum

    ones_t = dw.tile([P, 1], F32)
    nc.gpsimd.memset(ones_t[:], 1.0)
    st["ones"] = ones_t

    w1_s = dw.tile([128, 54 * 128], F32)
    nc.sync.dma_start(
        out=w1_s.rearrange("p (k i o) -> p k i o", k=27, i=2, o=128)[:],
        in_=c1wT_d.rearrange("k i p o -> p k i o"),
    )
    w2_s = dw.tile([128, 27 * 64], F32)
    nc.sync.dma_start(
        out=w2_s.rearrange("p (k o) -> p k o", k=27, o=64)[:],
        in_=c2wT_d.rearrange("k p o -> p k o"),
    )
    st["w2"] = w2_s
    for nm, d_, rows in (("c1b", c1b_d, 128), ("c2b", c2b_d, 64), ("lw", lw_d, 64)):
        t = dw.tile([rows, 1], F32, name=f"t_{nm}")
        nc.sync.dma_start(out=t[:], in_=d_[:])
        st[nm] = t
    st["lb_d"] = lb_d

    # padded conv1 input: [128, 6*6*6*2], (x, y, z, b) with batch innermost
    pad1 = {}
    for icb in range(2):
        pt = dact.tile([128, 432], F32, tag=f"pad1_{icb}", name=f"pad1_{icb}")
        nc.gpsimd.memset(pt[:], 0.0)
        for b in range(2):
            df = ddf.tile([128, 64], F32, tag="df", name=f"df_{b}_{icb}")
            nc.sync.dma_start(
                out=df[:],
                in_=domf_d[b * 256 + icb * 128 : b * 256 + icb * 128 + 128, :],
            )
            nc.scalar.copy(
                pt.rearrange("p (x y z b) -> p x y z b", x=6, y=6, z=6, b=2)[
                    :, 1:5, 1:5, 1:5, b
                ],
                df.rearrange("p (x y z) -> p x y z", x=4, y=4, z=4)[:],
            )
        pad1[icb] = pt

    # conv1: 54 accumulating matmuls -> PSUM [oc=128, (x,y,z,b) = 128]
    psum_c1 = dpsum.tile([128, 128], F32)
    st["psum_c1"] = psum_c1
    n_acc = 0
    for k in range(27):
        dx, dy, dz = k // 9, (k // 3) % 3, k % 3
        for icb in range(2):
            pv3 = pad1[icb].rearrange("p (x y zb) -> p x y zb", x=6, y=6, zb=12)
            nc.tensor.matmul(
                psum_c1[:],
                lhsT=w1_s[:, (k * 2 + icb) * 128 : (k * 2 + icb + 1) * 128],
                rhs=pv3[:, dx : dx + 4, dy : dy + 4, 2 * dz : 2 * dz + 8],
                start=(n_acc == 0),
                stop=(n_acc == 53),
            )
            n_acc += 1
    return st


def build_dom_tail(nc, tc, st, dom_d):
    """Pools, conv2, linear — emitted late so these DVE/ACT ops sit after the
    CAM ops in each engine's (in-order) stream."""
    dact, dpsum = st["dact"], st["dpsum"]
    psum_c1 = st["psum_c1"]

    # maxpool2 on PSUM views -> +bias -> relu -> padded conv2 input
    pad2 = dact.tile([128, 128], F32, tag="pad2")
    nc.gpsimd.memset(pad2[:], 0.0)
    for b in range(2):
        # psum_c1 col index = ((x*4+y)*4+z)*2+b; x = 2x'+a etc.
        pool_v = psum_c1.rearrange(
            "p (x a y b2 z c b) -> p x a y b2 z c b",
            x=2, a=2, y=2, b2=2, z=2, c=2, b=2,
        )
        mp = dact.tile([128, 8], F32, tag=f"mp1_{b}", name=f"mp1_{b}")
        mv = mp.rearrange("p (x y z) -> p x y z", x=2, y=2, z=2)
        first = True
        for da in range(2):
            for db in range(2):
                for dc in range(2):
                    v = pool_v[:, :, da, :, db, :, dc, b]
                    if first:
                        nc.vector.tensor_copy(mv[:], v)
                        first = False
                    else:
                        nc.vector.tensor_tensor(out=mv[:], in0=mv[:], in1=v, op=Alu.max)
        relu1 = dact.tile([128, 8], F32, tag=f"relu1_{b}", name=f"relu1_{b}")
        nc.scalar.activation(relu1[:], mp[:], Act.Relu, bias=st["c1b"][:, 0:1], scale=1.0)
        nc.scalar.copy(
            pad2.rearrange("p (x y z b) -> p x y z b", x=4, y=4, z=4, b=2)[
                :, 1:3, 1:3, 1:3, b
            ],
            relu1.rearrange("p (x y z) -> p x y z", x=2, y=2, z=2)[:],
        )

    # conv2: 27 accumulating matmuls -> PSUM [oc=64, (x,y,z,b) = 16]
    psum_c2 = dpsum.tile([64, 16], F32)
    pv2 = pad2.rearrange("p (x y zb) -> p x y zb", x=4, y=4, zb=8)
    for k in range(27):
        dx, dy, dz = k // 9, (k // 3) % 3, k % 3
        nc.tensor.matmul(
            psum_c2[:],
            lhsT=st["w2"][:, k * 64 : (k + 1) * 64],
            rhs=pv2[:, dx : dx + 2, dy : dy + 2, 2 * dz : 2 * dz + 4],
            start=(k == 0),
            stop=(k == 26),
        )

    # maxpool over 8 voxels -> +bias -> relu -> x2 * lin_w
    xw = dact.tile([128, 2], F32, tag="xw")
    for b in range(2):
        mp2 = dact.tile([64, 1], F32, tag=f"mp2_{b}", name=f"mp2_{b}")
        nc.vector.tensor_reduce(
            out=mp2[:],
            in_=psum_c2.rearrange("p (v b) -> p v b", v=8, b=2)[:, :, b],
            axis=mybir.AxisListType.X, op=Alu.max,
        )
        x2 = dact.tile([64, 1], F32, tag=f"x2_{b}", name=f"x2_{b}")
        nc.scalar.activation(x2[:], mp2[:], Act.Relu, bias=st["c2b"][:, 0:1], scale=1.0)
        nc.vector.tensor_tensor(
            out=xw[0:64, b : b + 1], in0=x2[:], in1=st["lw"][:], op=Alu.mult
        )
    # bias row: xw[64, b] = lin_b
    nc.gpsimd.dma_start(out=xw[64:65, 0:1], in_=st["lb_d"][:])
    nc.gpsimd.dma_start(out=xw[64:65, 1:2], in_=st["lb_d"][:])

    psd = dpsum.tile([2, 1], F32, tag="psd")
    nc.tensor.matmul(psd[:], lhsT=xw[0:65, 0:2], rhs=st["ones"][0:65, 0:1], start=True, stop=True)
    dom_s = dact.tile([2, 1], F32, tag="dom_s")
    nc.scalar.copy(dom_s[:], psd[:])
    nc.sync.dma_start(out=dom_d[:], in_=dom_s[:])


_NC_CACHE = None


def _get_nc():
    global _NC_CACHE
    if _NC_CACHE is None:
        _NC_CACHE = build_program()
    return _NC_CACHE


def make_in_maps(inputs):
    feature = np.ascontiguousarray(np.asarray(inputs["feature"], dtype=np.float32))
    masks = np.ascontiguousarray(np.asarray(inputs["masks"], dtype=np.int32))
    c1w = np.asarray(inputs["conv1_w"], np.float32).reshape(128, 256, 27)
    c2w = np.asarray(inputs["conv2_w"], np.float32).reshape(64, 128, 27)
    shared = {
        "dom_feat": np.asarray(inputs["dom_feat"], np.float32).reshape(512, 64),
        # lhsT layout [k, icb, ic, oc] / [k, ic, oc]
        "conv1_wT": np.ascontiguousarray(c1w.transpose(2, 1, 0)).reshape(27, 2, 128, 128),
        "conv2_wT": np.ascontiguousarray(c2w.transpose(2, 1, 0)),
        "conv1_b": np.asarray(inputs["conv1_b"], np.float32).reshape(128, 1),
        "conv2_b": np.asarray(inputs["conv2_b"], np.float32).reshape(64, 1),
        "lin_w": np.asarray(inputs["lin_w"], np.float32).reshape(64, 1),
        "lin_b": np.asarray(inputs["lin_b"], np.float32).reshape(1, 1),
    }
    in_maps = []
    for core in range(N_CORES):
        b, q = divmod(core, 4)
        dsl = slice(q * 24, (q + 1) * 24)
        in_maps.append(
            {
                "feat": np.ascontiguousarray(feature[b, 1:, dsl]).reshape(NCH, P, FT),
                "masks": np.ascontiguousarray(masks[b, 0, dsl]).reshape(P, FT),
                **shared,
            }
        )
    return in_maps


def assemble(results):
    cam = np.empty((2, NCH, 96, 96, 96), np.float32)
    pseudo = np.empty((2, 1, 96, 96, 96), np.int32)
    upd = np.empty((2, 1, 96, 96, 96), np.int32)
    for core in range(N_CORES):
        b, q = divmod(core, 4)
        dsl = slice(q * 24, (q + 1) * 24)
        r = results[core]
        cam[b, :, dsl] = np.asarray(r["cam"]).reshape(NCH, 24, 96, 96)
        pseudo[b, 0, dsl] = np.asarray(r["pseudo"]).reshape(24, 96, 96)
        upd[b, 0, dsl] = np.asarray(r["upd"]).reshape(24, 96, 96)
    dom = np.asarray(results[0]["dom"]).reshape(2, 1)
    return cam, pseudo, upd, dom


def _run(inputs, trace=False):
    nc = _get_nc()
    in_maps = make_in_maps(inputs)
    res = run_bass_kernel_spmd(nc, in_maps, list(range(N_CORES)), trace=trace)
    return assemble(res.results), res.exec_time_ns


def kernel(**inputs):
    out, _ = _run(inputs, trace=False)
    return out


# revision 16
# speedup vs baseline: 1.8457x; 1.0337x over previous
"""Trainium2 Bass kernel for CAMPseudoLabel.

Math (from the reference):
  cam    = relu(feature[:, 1:] / 96**3);  cam = cam * (cam > 0.2)
  pseudo = argmax(cam, axis=1) (first occurrence), int32
  upd    = masks + pseudo * (masks == 0)
  dom    = tiny 2-layer conv3d classifier + linear on dom_feat

Sharding: 8 cores = batch(2) x depth-quarters(4) of the 96^3 volume.
Per core: feature slice [13, 24, 96, 96] -> [13, 128, 1728] (channel 0 of the
original 14 is dropped by the reference before any use, so it is never sent).
The dom classifier is tiny; conv weights are pre-transposed on the host into
matmul (lhsT) layout and it is computed redundantly on every core, overlapped
with the CAM stream on the otherwise-idle tensor engine. Core 0's copy is
returned.
"""

import numpy as np

import concourse.bacc as bacc
import concourse.tile as tile
from concourse import mybir
from concourse.bass_utils import run_bass_kernel_spmd

F32 = mybir.dt.float32
I32 = mybir.dt.int32
Alu = mybir.AluOpType
Act = mybir.ActivationFunctionType

P = 128                 # SBUF partitions
FT = 1728               # free size per core: 24*96*96 / 128
NCH = 13                # cam channels (original channels 1..13)
GAMMA = 0.2
SCALE = float(np.float32(1.0) / np.float32(96 ** 3))  # f32(1/VOL), as jnp computes it
N_CORES = 8
# channels whose threshold op runs on gpsimd instead of DVE (load balance)
GP_TH_CHANNELS = frozenset()


def build_program():
    nc = bacc.Bacc("TRN2", target_bir_lowering=False, debug=False)

    # --- DRAM I/O (per core) ---
    feat_d = nc.declare_dram_parameter("feat", [NCH, P, FT], F32, isOutput=False)
    masks_d = nc.declare_dram_parameter("masks", [P, FT], I32, isOutput=False)
    domf_d = nc.declare_dram_parameter("dom_feat", [512, 64], F32, isOutput=False)
    # host-pretransposed conv weights, lhsT layout: [k, (icb,) ic, oc]
    c1wT_d = nc.declare_dram_parameter("conv1_wT", [27, 2, 128, 128], F32, isOutput=False)
    c1b_d = nc.declare_dram_parameter("conv1_b", [128, 1], F32, isOutput=False)
    c2wT_d = nc.declare_dram_parameter("conv2_wT", [27, 128, 64], F32, isOutput=False)
    c2b_d = nc.declare_dram_parameter("conv2_b", [64, 1], F32, isOutput=False)
    lw_d = nc.declare_dram_parameter("lin_w", [64, 1], F32, isOutput=False)
    lb_d = nc.declare_dram_parameter("lin_b", [1, 1], F32, isOutput=False)

    cam_d = nc.declare_dram_parameter("cam", [NCH, P, FT], F32, isOutput=True)
    pseudo_d = nc.declare_dram_parameter("pseudo", [P, FT], I32, isOutput=True)
    upd_d = nc.declare_dram_parameter("upd", [P, FT], I32, isOutput=True)
    dom_d = nc.declare_dram_parameter("dom", [2, 1], F32, isOutput=True)

    from contextlib import ExitStack
    with tile.TileContext(nc) as tc, ExitStack() as ctx:
        # dom conv1 first in program order: PE work overlaps the CAM stream;
        # its DVE/ACT tail is emitted near the end of the CAM channel loop
        dom_st = build_dom_head(nc, tc, ctx, domf_d, c1wT_d, c1b_d, c2wT_d,
                                c2b_d, lw_d, lb_d)
        build_cam(nc, tc, feat_d, masks_d, cam_d, pseudo_d, upd_d,
                  dom_tail=lambda: build_dom_tail(nc, tc, dom_st, dom_d),
                  dom_weights=dom_st["weights_cb"])
    nc.finalize()
    return nc


def build_cam(nc, tc, feat_d, masks_d, cam_d, pseudo_d, upd_d, dom_tail=None, dom_weights=None):
    BF16 = mybir.dt.bfloat16
    with (
        tc.tile_pool(name="featp", bufs=3) as featp,
        tc.tile_pool(name="sp", bufs=3) as sp,
        tc.tile_pool(name="thp", bufs=3) as thp,
        tc.tile_pool(name="kbp", bufs=3) as kbp,
        tc.tile_pool(name="gtp", bufs=2) as gtp,
        tc.tile_pool(name="state", bufs=1) as state,
    ):
        zero_i = state.tile([P, 1], I32)
        nc.gpsimd.memset(zero_i[:], 0)
        neg_gamma = state.tile([P, 1], F32)
        nc.gpsimd.memset(neg_gamma[:], -GAMMA)

        # The argmax runs on a bf16 key kb = bf16(relu(s - gamma)), which is an
        # order-isomorphic transform of the thresholded cam (exact ties
        # preserved; the downcast is monotone). bf16 doubles DVE throughput.
        idx = state.tile([P, FT], BF16)   # running argmax (values 0..12, exact)
        nc.gpsimd.memset(idx[:], 0)
        best = state.tile([P, FT], BF16)  # running key max

        for c in range(NCH):
            ft = featp.tile([P, FT], F32)
            nc.sync.dma_start(out=ft[:], in_=feat_d[c])
            s = sp.tile([P, FT], F32)
            nc.scalar.activation(s[:], ft[:], Act.Copy, bias=0.0, scale=SCALE)
            th = thp.tile([P, FT], F32)
            # th = (s > gamma) * s   (thresholded cam channel, exact f32)
            nc.vector.scalar_tensor_tensor(
                out=th[:], in0=s[:], scalar=GAMMA, in1=s[:],
                op0=Alu.is_gt, op1=Alu.mult,
            )
            nc.sync.dma_start(out=cam_d[c], in_=th[:])
            if c == 0:
                nc.scalar.activation(best[:], ft[:], Act.Relu, bias=neg_gamma[:, 0:1], scale=SCALE)
                if dom_weights is not None:
                    dom_weights()
            else:
                kb = kbp.tile([P, FT], BF16)
                nc.scalar.activation(kb[:], ft[:], Act.Relu, bias=neg_gamma[:, 0:1], scale=SCALE)
                gt = gtp.tile([P, FT], BF16)
                nc.vector.tensor_tensor(out=gt[:], in0=kb[:], in1=best[:], op=Alu.is_gt)
                # idx = max(idx, gt * c): strictly-greater keeps first occurrence.
                # (split ops: tensor_scalar runs 4x in bf16, tensor_tensor 2x —
                # a fused scalar_tensor_tensor would run 1x)
                nc.vector.tensor_scalar_mul(out=gt[:], in0=gt[:], scalar1=float(c))
                nc.vector.tensor_tensor(out=idx[:], in0=idx[:], in1=gt[:], op=Alu.max)
                if c < NCH - 1:
                    nc.vector.tensor_tensor(out=best[:], in0=best[:], in1=kb[:], op=Alu.max)
            if c == NCH - 2 and dom_tail is not None:
                dom_tail()

        pseudoi = state.tile([P, FT], I32)
        nc.scalar.copy(pseudoi[:], idx[:])
        nc.sync.dma_start(out=pseudo_d[:], in_=pseudoi[:])

        mk = state.tile([P, FT], I32)
        nc.sync.dma_start(out=mk[:], in_=masks_d[:])
        contrib = state.tile([P, FT], I32)
        # contrib = (masks == 0) * idx
        nc.vector.scalar_tensor_tensor(
            out=contrib[:], in0=mk[:], scalar=zero_i[:, 0:1], in1=pseudoi[:],
            op0=Alu.is_equal, op1=Alu.mult,
        )
        updt = state.tile([P, FT], I32)
        nc.vector.tensor_tensor(out=updt[:], in0=mk[:], in1=contrib[:], op=Alu.add)
        nc.sync.dma_start(out=upd_d[:], in_=updt[:])


def build_dom_head(nc, tc, ctx, domf_d, c1wT_d, c1b_d, c2wT_d, c2b_d, lw_d, lb_d):
    """Loads + padded inputs + conv1 matmuls (PE only; no DVE/ACT ops that
    could block the CAM stream). Batch goes innermost in the padded volumes so
    (z, b) merge into one AP dim and both batches share each matmul."""
    st = {}
    dw = ctx.enter_context(tc.tile_pool(name="dw", bufs=1))
    dact = ctx.enter_context(tc.tile_pool(name="dact", bufs=1))
    ddf = ctx.enter_context(tc.tile_pool(name="ddf", bufs=4))
    dpsum = ctx.enter_context(tc.tile_pool(name="dpsum", bufs=1, space="PSUM"))
    st["dact"], st["dpsum"] = dact, dpsum

    ones_t = dw.tile([P, 1], F32)
    nc.gpsimd.memset(ones_t[:], 1.0)
    st["ones"] = ones_t

    # padded conv1 input: [128, 6*6*6*2], (x, y, z, b) with batch innermost.
    # The tiny df DMAs are issued BEFORE the 4.4MB of weights so the engine
    # streams (ACT pad copies -> everything else) aren't stuck behind them.
    pad1 = {}
    for icb in range(2):
        pt = dact.tile([128, 432], F32, tag=f"pad1_{icb}", name=f"pad1_{icb}")
        nc.gpsimd.memset(pt[:], 0.0)
        for b in range(2):
            df = ddf.tile([128, 64], F32, tag="df", name=f"df_{b}_{icb}")
            nc.sync.dma_start(
                out=df[:],
                in_=domf_d[b * 256 + icb * 128 : b * 256 + icb * 128 + 128, :],
            )
            nc.scalar.copy(
                pt.rearrange("p (x y z b) -> p x y z b", x=6, y=6, z=6, b=2)[
                    :, 1:5, 1:5, 1:5, b
                ],
                df.rearrange("p (x y z) -> p x y z", x=4, y=4, z=4)[:],
            )
        pad1[icb] = pt
    st["pad1"] = pad1

    def dom_weights_and_conv1():
        w1_s = dw.tile([128, 54 * 128], F32, name="w1_s")
        nc.sync.dma_start(
            out=w1_s.rearrange("p (k i o) -> p k i o", k=27, i=2, o=128)[:],
            in_=c1wT_d.rearrange("k i p o -> p k i o"),
        )
        w2_s = dw.tile([128, 27 * 64], F32, name="w2_s")
        nc.sync.dma_start(
            out=w2_s.rearrange("p (k o) -> p k o", k=27, o=64)[:],
            in_=c2wT_d.rearrange("k p o -> p k o"),
        )
        st["w2"] = w2_s
        for nm, d_, rows in (("c1b", c1b_d, 128), ("c2b", c2b_d, 64), ("lw", lw_d, 64)):
            t = dw.tile([rows, 1], F32, name=f"t_{nm}")
            nc.sync.dma_start(out=t[:], in_=d_[:])
            st[nm] = t
        st["lb_d"] = lb_d

        # conv1: 54 accumulating matmuls -> PSUM [oc=128, (x,y,z,b) = 128]
        psum_c1 = dpsum.tile([128, 128], F32, name="psum_c1")
        st["psum_c1"] = psum_c1
        n_acc = 0
        for k in range(27):
            dx, dy, dz = k // 9, (k // 3) % 3, k % 3
            for icb in range(2):
                pv3 = pad1[icb].rearrange("p (x y zb) -> p x y zb", x=6, y=6, zb=12)
                nc.tensor.matmul(
                    psum_c1[:],
                    lhsT=w1_s[:, (k * 2 + icb) * 128 : (k * 2 + icb + 1) * 128],
                    rhs=pv3[:, dx : dx + 4, dy : dy + 4, 2 * dz : 2 * dz + 8],
                    start=(n_acc == 0),
                    stop=(n_acc == 53),
                )
                n_acc += 1

    st["weights_cb"] = dom_weights_and_conv1
    return st


def build_dom_tail(nc, tc, st, dom_d):
    """Pools, conv2, linear — emitted late so these DVE/ACT ops sit after the
    CAM ops in each engine's (in-order) stream."""
    dact, dpsum = st["dact"], st["dpsum"]
    psum_c1 = st["psum_c1"]

    # maxpool2 on PSUM views -> +bias -> relu -> padded conv2 input
    pad2 = dact.tile([128, 128], F32, tag="pad2")
    nc.gpsimd.memset(pad2[:], 0.0)
    for b in range(2):
        # psum_c1 col index = ((x*4+y)*4+z)*2+b; x = 2x'+a etc.
        pool_v = psum_c1.rearrange(
            "p (x a y b2 z c b) -> p x a y b2 z c b",
            x=2, a=2, y=2, b2=2, z=2, c=2, b=2,
        )
        mp = dact.tile([128, 8], F32, tag=f"mp1_{b}", name=f"mp1_{b}")
        mv = mp.rearrange("p (x y z) -> p x y z", x=2, y=2, z=2)
        first = True
        for da in range(2):
            for db in range(2):
                for dc in range(2):
                    v = pool_v[:, :, da, :, db, :, dc, b]
                    if first:
                        nc.vector.tensor_copy(mv[:], v)
                        first = False
                    else:
                        nc.vector.tensor_tensor(out=mv[:], in0=mv[:], in1=v, op=Alu.max)
        relu1 = dact.tile([128, 8], F32, tag=f"relu1_{b}", name=f"relu1_{b}")
        nc.scalar.activation(relu1[:], mp[:], Act.Relu, bias=st["c1b"][:, 0:1], scale=1.0)
        nc.scalar.copy(
            pad2.rearrange("p (x y z b) -> p x y z b", x=4, y=4, z=4, b=2)[
                :, 1:3, 1:3, 1:3, b
            ],
            relu1.rearrange("p (x y z) -> p x y z", x=2, y=2, z=2)[:],
        )

    # conv2: 27 accumulating matmuls -> PSUM [oc=64, (x,y,z,b) = 16]
    psum_c2 = dpsum.tile([64, 16], F32)
    pv2 = pad2.rearrange("p (x y zb) -> p x y zb", x=4, y=4, zb=8)
    for k in range(27):
        dx, dy, dz = k // 9, (k // 3) % 3, k % 3
        nc.tensor.matmul(
            psum_c2[:],
            lhsT=st["w2"][:, k * 64 : (k + 1) * 64],
            rhs=pv2[:, dx : dx + 2, dy : dy + 2, 2 * dz : 2 * dz + 4],
            start=(k == 0),
            stop=(k == 26),
        )

    # maxpool over 8 voxels -> +bias -> relu -> x2 * lin_w
    xw = dact.tile([128, 2], F32, tag="xw")
    for b in range(2):
        mp2 = dact.tile([64, 1], F32, tag=f"mp2_{b}", name=f"mp2_{b}")
        nc.vector.tensor_reduce(
            out=mp2[:],
            in_=psum_c2.rearrange("p (v b) -> p v b", v=8, b=2)[:, :, b],
            axis=mybir.AxisListType.X, op=Alu.max,
        )
        x2 = dact.tile([64, 1], F32, tag=f"x2_{b}", name=f"x2_{b}")
        nc.scalar.activation(x2[:], mp2[:], Act.Relu, bias=st["c2b"][:, 0:1], scale=1.0)
        nc.vector.tensor_tensor(
            out=xw[0:64, b : b + 1], in0=x2[:], in1=st["lw"][:], op=Alu.mult
        )
    # bias row: xw[64, b] = lin_b
    nc.gpsimd.dma_start(out=xw[64:65, 0:1], in_=st["lb_d"][:])
    nc.gpsimd.dma_start(out=xw[64:65, 1:2], in_=st["lb_d"][:])

    psd = dpsum.tile([2, 1], F32, tag="psd")
    nc.tensor.matmul(psd[:], lhsT=xw[0:65, 0:2], rhs=st["ones"][0:65, 0:1], start=True, stop=True)
    dom_s = dact.tile([2, 1], F32, tag="dom_s")
    nc.scalar.copy(dom_s[:], psd[:])
    nc.sync.dma_start(out=dom_d[:], in_=dom_s[:])


_NC_CACHE = None


def _get_nc():
    global _NC_CACHE
    if _NC_CACHE is None:
        _NC_CACHE = build_program()
    return _NC_CACHE


def make_in_maps(inputs):
    feature = np.ascontiguousarray(np.asarray(inputs["feature"], dtype=np.float32))
    masks = np.ascontiguousarray(np.asarray(inputs["masks"], dtype=np.int32))
    c1w = np.asarray(inputs["conv1_w"], np.float32).reshape(128, 256, 27)
    c2w = np.asarray(inputs["conv2_w"], np.float32).reshape(64, 128, 27)
    shared = {
        "dom_feat": np.asarray(inputs["dom_feat"], np.float32).reshape(512, 64),
        # lhsT layout [k, icb, ic, oc] / [k, ic, oc]
        "conv1_wT": np.ascontiguousarray(c1w.transpose(2, 1, 0)).reshape(27, 2, 128, 128),
        "conv2_wT": np.ascontiguousarray(c2w.transpose(2, 1, 0)),
        "conv1_b": np.asarray(inputs["conv1_b"], np.float32).reshape(128, 1),
        "conv2_b": np.asarray(inputs["conv2_b"], np.float32).reshape(64, 1),
        "lin_w": np.asarray(inputs["lin_w"], np.float32).reshape(64, 1),
        "lin_b": np.asarray(inputs["lin_b"], np.float32).reshape(1, 1),
    }
    in_maps = []
    for core in range(N_CORES):
        b, q = divmod(core, 4)
        dsl = slice(q * 24, (q + 1) * 24)
        in_maps.append(
            {
                "feat": np.ascontiguousarray(feature[b, 1:, dsl]).reshape(NCH, P, FT),
                "masks": np.ascontiguousarray(masks[b, 0, dsl]).reshape(P, FT),
                **shared,
            }
        )
    return in_maps


def assemble(results):
    cam = np.empty((2, NCH, 96, 96, 96), np.float32)
    pseudo = np.empty((2, 1, 96, 96, 96), np.int32)
    upd = np.empty((2, 1, 96, 96, 96), np.int32)
    for core in range(N_CORES):
        b, q = divmod(core, 4)
        dsl = slice(q * 24, (q + 1) * 24)
        r = results[core]
        cam[b, :, dsl] = np.asarray(r["cam"]).reshape(NCH, 24, 96, 96)
        pseudo[b, 0, dsl] = np.asarray(r["pseudo"]).reshape(24, 96, 96)
        upd[b, 0, dsl] = np.asarray(r["upd"]).reshape(24, 96, 96)
    dom = np.asarray(results[0]["dom"]).reshape(2, 1)
    return cam, pseudo, upd, dom


def _run(inputs, trace=False):
    nc = _get_nc()
    in_maps = make_in_maps(inputs)
    res = run_bass_kernel_spmd(nc, in_maps, list(range(N_CORES)), trace=trace)
    return assemble(res.results), res.exec_time_ns


def kernel(**inputs):
    out, _ = _run(inputs, trace=False)
    return out
